# revision 1
# baseline (speedup 1.0000x reference)
"""Trainium2 Bass kernel for nn_AttentionManifold (B=32, P=128, IN=64, OUT=32).

Data-parallel over batch: each of 8 NeuronCores handles 4 batches.
Per core:
  A. Q/K/V = W x W^T: shared-stationary f32r matmuls + DVE 32x32 block
     transpose between the two contractions. Kinds (q,k,v) on partition
     strips 0-31/32-63/64-95.
  B. logm via inverse scaling-squaring: 2 scaled Newton-Schulz sqrt stages
     (deferred-scalar form, offline-tuned alphas, global normalizer
     c=8.5 folded into compile-time scalars) + degree-11 log series.
     Per-sample 32x32 matmuls on tile_position diagonal strips.
     log A = 4*p(E) + ln(c) I, with the constant diagonal term added via a
     host-provided (const * I) tile.
  C. attention: Gram via 32 per-j accumulating matmuls in [key, query]
     layout, qq/kk via ones-matmul broadcasts, softmax along free axis.
  D. Frechet mean: mean_logT = LVflat^T @ S^T chunks; expm via
     scaling-squaring (Taylor d=8, 5 squarings) on 4-sample strips.
"""
import math
import numpy as np

import concourse.bacc as bacc
import concourse.mybir as mybir
import concourse.tile as tile
from concourse.bass_utils import run_bass_kernel_spmd

F32 = mybir.dt.float32
F32R = mybir.dt.float32r
MULT = mybir.AluOpType.mult
ADD = mybir.AluOpType.add

B, P, IN = 32, 128, 64
NCORES = 8
BLOC = B // NCORES
GRP = 16
NGRP = P // GRP

CGLOB = 8.5                      # global SPD normalizer, > lambda_max (~7.3)
ALPHAS0 = [1.7939874036898087, 1.6696029929467766, 1.5753856846965621,
           1.3802459084155867, 1.1355312114962206, 1.0145731825395088,
           1.0001600783454123]
ALPHAS1 = [1.639353436157538, 1.3943732234795634, 1.1476361656772485,
           1.0173994934181363, 1.000228417137108]
# log series coeffs on M in [0.1627, 0.9658]: log(M) ~ sum_k SER[k] (M-I)^k
SER = [0.00025761896563381015, 1.016394391935819, -0.08934176002367167,
       5.76267183490063, 42.59363464146395, 215.5576662374658,
       713.4419495013208, 1577.6143678674662, 2302.793898554353,
       2133.2456306970385, 1137.177063455271, 266.64841671372346]
SDEG = len(SER) - 1              # 11
POW2S = 4.0
LNC_CONST = math.log(CGLOB) + POW2S * SER[0]
EXP_K = 5
EXP_D = 8
EXP_C = [1.0 / math.factorial(k) for k in range(EXP_D + 1)]


def _stage_scalars():
    out = []
    for st, alphas in enumerate((ALPHAS0, ALPHAS1)):
        gammas = []
        p = (1.0 / CGLOB) if st == 0 else 1.0
        q = 1.0
        for a in alphas:
            gammas.append(-(a * a / 3.0) * (p * q))
            p *= 1.5 * a
            q *= 1.5 * a
        out.append((gammas, p))
    return out

STAGE_SCALARS = _stage_scalars()


def _persample_round(nc, out_ap_fn, lhs_fn, rhs_fn, nsamp, tile_col_fn=None):
    for w in range(3):
        for s in range(nsamp):
            tc_ = (32 * w, 32 * w) if tile_col_fn is None else (32 * w, tile_col_fn(w))
            nc.tensor.matmul(out_ap_fn(w, s), lhs_fn(w, s), rhs_fn(w, s),
                             start=True, stop=True, tile_position=tc_)


def build_nc(debug_stage=99):
    nc = bacc.Bacc("TRN2", target_bir_lowering=False, debug=False,
                   num_devices=NCORES)
    x_in = nc.dram_tensor("x", [BLOC, P, IN, IN], F32, kind="ExternalInput").ap()
    wallT_in = nc.dram_tensor("wallT", [IN, 96], F32, kind="ExternalInput").ap()
    wall2_in = nc.dram_tensor("wall2", [96, 64], F32, kind="ExternalInput").ap()
    ibig_in = nc.dram_tensor("ibig", [128, GRP, 32], F32, kind="ExternalInput").ap()
    ibigl_in = nc.dram_tensor("ibigl", [32, 3 * GRP * 32], F32, kind="ExternalInput").ap()
    ibgx_in = nc.dram_tensor("ibgx", [128, 32, 32], F32, kind="ExternalInput").ap()
    id128_in = nc.dram_tensor("id128", [128, 128], F32, kind="ExternalInput").ap()
    ones_in = nc.dram_tensor("onesc", [96, 128], F32, kind="ExternalInput").ap()
    out_d = nc.dram_tensor("out", [BLOC, P, 32, 32], F32, kind="ExternalOutput").ap()

    dbg = {}
    def dbg_out(name, shape):
        dbg[name] = nc.dram_tensor(name, shape, F32, kind="ExternalOutput").ap()
    if debug_stage == 1:
        dbg_out("d_mats", [BLOC, 96, P, 32])
    if debug_stage == 2:
        dbg_out("d_mhat", [BLOC, 96, P, 32])
        dbg_out("d_mats", [BLOC, 96, P, 32])
        dbg_out("d_m0", [BLOC, 96, P, 32])
        dbg_out("d_c0", [BLOC, 96, P, 32])
        dbg_out("d_yz1", [BLOC, 96, P, 64])
    if debug_stage == 3:
        dbg_out("d_lf", [BLOC, 32, 3, P, 32])
        dbg_out("d_mats", [BLOC, 96, P, 32])
    if debug_stage in (5, 6, 7):
        dbg_out("d_gt", [BLOC, 128, 32, 32])
    if debug_stage == 5:
        dbg_out("d_tay", [BLOC, 128, 32, 32])
        dbg_out("d_mlfs", [BLOC, 128, 1024])
    if debug_stage == 4:
        dbg_out("d_en", [BLOC, 128, 128])
        dbg_out("d_s", [BLOC, 128, 128])
        dbg_out("d_mlfs", [BLOC, 128, 1024])

    with tile.TileContext(nc) as tc:
        with (
            tc.tile_pool(name="const", bufs=1) as cpool,
            tc.tile_pool(name="perb", bufs=1) as bpool,
            tc.tile_pool(name="grp", bufs=2) as gpool,
            tc.tile_pool(name="ps", bufs=1, space="PSUM") as ps,
            tc.tile_pool(name="dscr", bufs=1, space="DRAM") as dpool,
        ):
            scrV_t = dpool.tile([32, P, 32], F32, name="scrV")
            scrM_t = dpool.tile([P, 1024], F32, name="scrM")
            scrV = scrV_t[:]
            scrM = scrM_t[:]
            wallT = cpool.tile([IN, 96], F32)
            nc.sync.dma_start(wallT[:], wallT_in[:])
            wallTr = cpool.tile([IN, 96], F32R)
            nc.vector.tensor_copy(wallTr[:], wallT[:])
            wall2 = cpool.tile([96, 2, 32], F32)
            nc.sync.dma_start(wall2[:], wall2_in.rearrange("p (h j) -> p h j", h=2))
            ibig = cpool.tile([128, GRP, 32], F32)
            nc.sync.dma_start(ibig[:], ibig_in[:])
            ibigl = cpool.tile([32, 3, GRP, 32], F32)
            nc.sync.dma_start(ibigl[:], ibigl_in.rearrange(
                "p (w s j) -> p w s j", w=3, s=GRP))
            ibgx = cpool.tile([128, 32, 32], F32)
            nc.sync.dma_start(ibgx[:], ibgx_in[:])
            id128 = cpool.tile([128, 128], F32)
            nc.sync.dma_start(id128[:], id128_in[:])
            onesc = cpool.tile([96, 128], F32)
            nc.sync.dma_start(onesc[:], ones_in[:])

            for b in range(BLOC):
                # ================= stage A =================
                xt = bpool.tile([IN, P, IN], F32, tag="xt", bufs=1)
                nc.sync.dma_start(xt[:], x_in[b].rearrange("p i j -> i p j"))
                ytT = bpool.tile([96, P, 2, 32], F32, tag="ytT", bufs=1)
                for t in range(16):
                    xs = xt[:].rearrange("i p j -> i (p j)")[:, 512 * t:512 * (t + 1)]
                    xr = gpool.tile([IN, 512], F32R, tag="xr")
                    nc.vector.tensor_copy(xr[:], xs)
                    psY = ps.tile([96, 512], F32, tag="p1a", name=f"psY{b}_{t}")
                    nc.tensor.matmul(psY[:], wallTr[:], xr[:], start=True, stop=True)
                    nc.vector.transpose(
                        ytT[:].rearrange("p m h j -> p (m h j)")[:, 512 * t:512 * (t + 1)],
                        psY[:])
                mats = bpool.tile([96, P, 32], F32, tag="mats", bufs=1)
                for g in range(NGRP):
                    gsl = slice(GRP * g, GRP * (g + 1))
                    psQ = ps.tile([96, GRP, 32], F32, tag="p1b", name=f"psQ{b}_{g}")
                    for h in range(2):
                        for w in range(3):
                            sl = slice(32 * w, 32 * w + 32)
                            nc.tensor.matmul(
                                psQ[sl, :, :], wall2[sl, h, :],
                                ytT[sl, gsl, h, :],
                                start=(h == 0), stop=(h == 1),
                                tile_position=(32 * w, 32 * w))
                    nc.scalar.copy(mats[:, gsl, :], psQ[:])
                if "d_mats" in dbg:
                    nc.sync.dma_start(dbg["d_mats"][b], mats[:])
                if debug_stage <= 1:
                    continue

                # ================= stage B: logm =================
                lf = (bpool.tile([32, 3, P, 32], F32, tag="lf", bufs=1,
                                 name="lf")
                      if debug_stage != 2 else None)
                mhat_dbg = (bpool.tile([96, P, 32], F32, tag="mhat", bufs=1,
                                        name="mhat_dbg")
                            if "d_mhat" in dbg else None)
                m0_dbg = (bpool.tile([96, P, 32], F32, tag="m0d", bufs=1,
                                     name="m0_dbg")
                          if "d_m0" in dbg else None)
                yz1_dbg = (bpool.tile([96, P, 64], F32, tag="yz1d", bufs=1,
                                      name="yz1_dbg")
                           if "d_yz1" in dbg else None)

                c0_dbg = (bpool.tile([96, P, 32], F32, tag="c0d", bufs=1,
                                     name="c0_dbg")
                          if "d_c0" in dbg else None)
                for g in range(NGRP):
                    gsl = slice(GRP * g, GRP * (g + 1))
                    yz = gpool.tile([96, GRP, 64], F32, tag="yz")
                    mcur = gpool.tile([96, GRP, 32], F32, tag="mcur")
                    nc.vector.tensor_copy(mcur[:], mats[:, gsl, :])
                    ctile = gpool.tile([96, GRP, 32], F32, tag="ctile")
                    for st in range(2):
                        gammas, pn = STAGE_SCALARS[st]
                        nc.vector.scalar_tensor_tensor(
                            ctile[:], mcur[:], float(gammas[0]), ibig[0:96],
                            op0=MULT, op1=ADD)
                        if st == 0 and c0_dbg is not None:
                            nc.vector.tensor_copy(c0_dbg[:, gsl, :], ctile[:])
                        psYZ = ps.tile([96, GRP, 64], F32, tag="p2a",
                                       name=f"psYZ0_{b}_{g}_{st}")
                        _persample_round(
                            nc,
                            lambda w, s: psYZ[32 * w:32 * w + 32, s, 0:32],
                            lambda w, s: ctile[32 * w:32 * w + 32, s, :],
                            lambda w, s: mcur[32 * w:32 * w + 32, s, :], GRP)
                        nc.scalar.copy(yz[:, :, 0:32], psYZ[:, :, 0:32])
                        nc.vector.tensor_copy(yz[:, :, 32:64], ctile[:])
                        for k in range(1, len(gammas)):
                            psT = ps.tile([96, GRP, 32], F32, tag="p1a",
                                          name=f"psT{b}_{g}_{st}_{k}")
                            _persample_round(
                                nc,
                                lambda w, s: psT[32 * w:32 * w + 32, s, :],
                                lambda w, s: yz[32 * w:32 * w + 32, s, 32:64],
                                lambda w, s: yz[32 * w:32 * w + 32, s, 0:32], GRP)
                            nc.vector.scalar_tensor_tensor(
                                ctile[:], psT[:], float(gammas[k]), ibig[0:96],
                                op0=MULT, op1=ADD)
                            psYZ = ps.tile([96, GRP, 64], F32, tag="p2a",
                                           name=f"psYZ{b}_{g}_{st}_{k}")
                            _persample_round(
                                nc,
                                lambda w, s: psYZ[32 * w:32 * w + 32, s, :],
                                lambda w, s: ctile[32 * w:32 * w + 32, s, :],
                                lambda w, s: yz[32 * w:32 * w + 32, s, :], GRP)
                            if k == len(gammas) - 1:
                                nc.scalar.mul(mcur[:], psYZ[:, :, 0:32], float(pn))
                            else:
                                nc.scalar.copy(yz[:], psYZ[:])
                            if st == 0 and k == 1 and yz1_dbg is not None:
                                nc.vector.tensor_copy(yz1_dbg[:, gsl, :], yz[:])

                        if st == 0 and m0_dbg is not None:
                            nc.vector.tensor_copy(m0_dbg[:, gsl, :], mcur[:])
                    if mhat_dbg is not None:
                        nc.vector.tensor_copy(mhat_dbg[:, gsl, :], mcur[:])
                        continue
                    # series (coeffs pre-scaled by 4 = 2^s)
                    etile = gpool.tile([96, GRP, 32], F32, tag="etile")
                    nc.vector.scalar_tensor_tensor(
                        etile[:], ibig[0:96], -1.0, mcur[:], op0=MULT, op1=ADD)
                    acc = gpool.tile([96, GRP, 32], F32, tag="acc")
                    nc.vector.tensor_scalar_mul(acc[:], ibig[0:96],
                                                float(POW2S * SER[SDEG]))
                    for k in range(SDEG - 1, 0, -1):
                        psH = ps.tile([96, GRP, 32], F32, tag="p1a",
                                      name=f"psH{b}_{g}_{k}")
                        _persample_round(
                            nc,
                            lambda w, s: psH[32 * w:32 * w + 32, s, :],
                            lambda w, s: etile[32 * w:32 * w + 32, s, :],
                            lambda w, s: acc[32 * w:32 * w + 32, s, :], GRP)
                        nc.vector.scalar_tensor_tensor(
                            acc[:], ibig[0:96], float(POW2S * SER[k]), psH[:],
                            op0=MULT, op1=ADD)
                    psL = ps.tile([32, 3, GRP, 32], F32, tag="p3",
                                  name=f"psL{b}_{g}")
                    _persample_round(
                        nc,
                        lambda w, s: psL[:, w, s, :],
                        lambda w, s: etile[32 * w:32 * w + 32, s, :],
                        lambda w, s: acc[32 * w:32 * w + 32, s, :], GRP,
                        tile_col_fn=lambda w: 0)
                    nc.vector.tensor_tensor(lf[:, :, gsl, :], psL[:], ibigl[:],
                                            op=ADD)
                if mhat_dbg is not None:
                    nc.sync.dma_start(dbg["d_mhat"][b], mhat_dbg[:])
                    if m0_dbg is not None:
                        nc.sync.dma_start(dbg["d_m0"][b], m0_dbg[:])
                    if c0_dbg is not None:
                        nc.sync.dma_start(dbg["d_c0"][b], c0_dbg[:])
                    if yz1_dbg is not None:
                        nc.sync.dma_start(dbg["d_yz1"][b], yz1_dbg[:])

                    continue
                if "d_lf" in dbg:
                    nc.sync.dma_start(dbg["d_lf"][b], lf[:])
                if debug_stage <= 3:
                    continue

                # ================= stage C: attention =================
                qrow = bpool.tile([1, 128], F32, tag="qrow", bufs=1)
                krow = bpool.tile([1, 128], F32, tag="krow", bufs=1)
                for kind, row in ((0, qrow), (1, krow)):
                    sqf = bpool.tile([32, P, 32], F32, tag="ytT", bufs=1)
                    nc.vector.tensor_tensor(sqf[:], lf[:, kind], lf[:, kind],
                                            op=MULT)
                    rsf = bpool.tile([32, P], F32, tag="rsf", bufs=1)
                    nc.vector.tensor_reduce(rsf[:], sqf[:],
                                            axis=mybir.AxisListType.X, op=ADD)
                    psq = ps.tile([1, 128], F32, tag="p1c",
                                  name=f"psq{b}_{kind}")
                    nc.tensor.matmul(psq[:], onesc[0:32, 0:1], rsf[:],
                                     start=True, stop=True)
                    nc.scalar.mul(row[:], psq[:], -0.5)
                psE = ps.tile([128, 128], F32, tag="p1c", name=f"psE{b}")
                for j in range(32):
                    nc.tensor.matmul(psE[:], lf[:, 1, :, j], lf[:, 0, :, j],
                                     start=(j == 0), stop=False)
                nc.tensor.matmul(psE[:], onesc[0:1, :], qrow[:],
                                 start=False, stop=False)
                nc.tensor.matmul(psE[:], krow[:], onesc[0:1, :],
                                 start=False, stop=True)
                w1 = bpool.tile([128, 128], F32, tag="w1", bufs=1)
                nc.scalar.activation(w1[:], psE[:],
                                     mybir.ActivationFunctionType.Relu,
                                     scale=-2.0)
                if "d_en" in dbg:
                    nc.sync.dma_start(dbg["d_en"][b], w1[:])
                w2 = bpool.tile([128, 128], F32, tag="w2", bufs=1)
                nc.scalar.activation(w2[:], w1[:],
                                     mybir.ActivationFunctionType.Ln, bias=1.0)
                nc.vector.tensor_scalar_add(w2[:], w2[:], 1.0)
                wr = bpool.tile([128, 128], F32, tag="wr", bufs=1)
                nc.vector.reciprocal(wr[:], w2[:])
                srow = bpool.tile([128, 1], F32, tag="srow", bufs=1)
                ew = bpool.tile([128, 128], F32, tag="ew", bufs=1)
                nc.scalar.activation(ew[:], wr[:],
                                     mybir.ActivationFunctionType.Exp,
                                     accum_out=srow[:])
                rsrow = bpool.tile([128, 1], F32, tag="rsrow", bufs=1)
                nc.vector.reciprocal(rsrow[:], srow[:])
                stile = bpool.tile([128, 128], F32, tag="stile", bufs=1)
                nc.scalar.mul(stile[:], ew[:], rsrow[:])
                if "d_s" in dbg:
                    nc.sync.dma_start(dbg["d_s"][b], stile[:])
                psST = ps.tile([128, 128], F32, tag="p1c", name=f"psST{b}")
                nc.tensor.transpose(psST[:], stile[:], id128[:])
                st_t = bpool.tile([128, 128], F32, tag="st_t", bufs=1)
                nc.scalar.copy(st_t[:], psST[:])
                lvfs = bpool.tile([128, 1024], F32, tag="lvfs", bufs=1)
                nc.sync.dma_start(scrV[:], lf[:, 2])
                nc.sync.dma_start(
                    lvfs[:].rearrange("m (i j) -> m i j", i=32),
                    scrV.rearrange("i m j -> m i j"))
                psML = ps.tile([128, 8, 128], F32, tag="p2a", name=f"psML{b}")
                for c in range(8):
                    nc.tensor.matmul(psML[:, c, :], lvfs[:, 128 * c:128 * (c + 1)],
                                     st_t[:], start=True, stop=True)
                mlT = bpool.tile([128, 8, 128], F32, tag="mlT", bufs=1)
                nc.scalar.copy(mlT[:], psML[:])
                psMT = ps.tile([128, 8, 128], F32, tag="p3", name=f"psMT{b}")
                for c in range(8):
                    nc.tensor.transpose(psMT[:, c, :], mlT[:, c, :], id128[:])
                mlfs = bpool.tile([128, 1024], F32, tag="mlfs", bufs=1)
                nc.scalar.mul(mlfs[:], psMT[:].rearrange("m c e -> m (c e)"),
                              1.0 / (2.0 ** EXP_K))
                if "d_mlfs" in dbg:
                    nc.sync.dma_start(dbg["d_mlfs"][b], mlfs[:])
                if debug_stage <= 4:
                    continue

                # ================= stage D: expm =================
                gt = bpool.tile([128, 32, 32], F32, tag="gt", bufs=1)
                nc.sync.dma_start(scrM[:], mlfs[:])
                for rr in range(4):
                    nc.sync.dma_start(
                        gt[32 * rr:32 * rr + 32, :, :],
                        scrM[rr::4, :].rearrange("g (i j) -> i g j", i=32))
                if "d_gt" in dbg:
                    nc.sync.dma_start(dbg["d_gt"][b], gt[:])
                if debug_stage == 6:
                    continue
                acx = bpool.tile([128, 32, 32], F32, tag="acx", bufs=1)
                nc.vector.tensor_scalar_mul(acx[:], ibgx[:], float(EXP_C[EXP_D]))
                for k in range(EXP_D - 1, -1, -1):
                    psX = ps.tile([128, 32, 32], F32, tag="p2a",
                                  name=f"psXh{b}_{k}")
                    for r in range(4):
                        sl = slice(32 * r, 32 * r + 32)
                        for s in range(32):
                            nc.tensor.matmul(psX[sl, s, :], gt[sl, s, :],
                                             acx[sl, s, :], start=True, stop=True,
                                             tile_position=(32 * r, 32 * r))
                    nc.vector.scalar_tensor_tensor(
                        acx[:], ibgx[:], float(EXP_C[k]), psX[:],
                        op0=MULT, op1=ADD)
                if "d_tay" in dbg:
                    nc.sync.dma_start(dbg["d_tay"][b], acx[:])
                if debug_stage == 7:
                    for rr in range(4):
                        nc.sync.dma_start(
                            out_d[b][rr::4].rearrange("g i j -> i g j"),
                            acx[32 * rr:32 * rr + 32, :, :])
                    continue
                for sq_i in range(EXP_K):
                    psX = ps.tile([128, 32, 32], F32, tag="p2a",
                                  name=f"psXs{b}_{sq_i}")
                    for r in range(4):
                        sl = slice(32 * r, 32 * r + 32)
                        for s in range(32):
                            nc.tensor.matmul(psX[sl, s, :], acx[sl, s, :],
                                             acx[sl, s, :], start=True, stop=True,
                                             tile_position=(32 * r, 32 * r))
                    nc.scalar.copy(acx[:], psX[:])
                for rr in range(4):
                    nc.sync.dma_start(
                        out_d[b][rr::4].rearrange("g i j -> i g j"),
                        acx[32 * rr:32 * rr + 32, :, :])
    nc.compile()
    return nc, dbg


def host_constants(Wq, Wk, Wv):
    wallT = np.concatenate([Wq.T, Wk.T, Wv.T], axis=1).astype(np.float32)
    def w2(W):
        WT = np.ascontiguousarray(W.T.astype(np.float32))
        return np.concatenate([WT[0:32], WT[32:64]], axis=1)
    wall2 = np.concatenate([w2(Wq), w2(Wk), w2(Wv)], axis=0)
    eye = np.eye(32, dtype=np.float32)
    ibig = np.broadcast_to(eye[None, :, None, :],
                           (4, 32, GRP, 32)).reshape(128, GRP, 32).copy()
    ibgx = np.broadcast_to(eye[None, :, None, :],
                           (4, 32, 32, 32)).reshape(128, 32, 32).copy()
    ibigl = (LNC_CONST * np.broadcast_to(
        eye[:, None, None, :], (32, 3, GRP, 32))).reshape(32, 3 * GRP * 32)
    ibigl = np.ascontiguousarray(ibigl, dtype=np.float32)
    id128 = np.eye(128, dtype=np.float32)
    onesc = np.ones((96, 128), dtype=np.float32)
    return {"wallT": wallT, "wall2": wall2, "ibig": ibig, "ibgx": ibgx,
            "ibigl": ibigl, "id128": id128, "onesc": onesc}


_NC_CACHE = {}

def kernel(x, Wq, Wk, Wv):
    if "full" not in _NC_CACHE:
        _NC_CACHE["full"] = build_nc(99)
    nc, _ = _NC_CACHE["full"]
    consts = host_constants(np.asarray(Wq), np.asarray(Wk), np.asarray(Wv))
    x = np.asarray(x, dtype=np.float32)
    in_maps = []
    for c in range(NCORES):
        m = {"x": np.ascontiguousarray(x[BLOC * c:BLOC * (c + 1)])}
        m.update(consts)
        in_maps.append(m)
    res = run_bass_kernel_spmd(nc, in_maps, list(range(NCORES)))
    out = np.concatenate([res.results[c]["out"] for c in range(NCORES)], axis=0)
    return out.astype(np.float32)



# revision 2
# speedup vs baseline: 1.0321x; 1.0321x over previous
"""Trainium2 Bass kernel for nn_AttentionManifold (B=32, P=128, IN=64, OUT=32).

Data-parallel over batch: each of 8 NeuronCores handles 4 batches.
Per core:
  A. Q/K/V = W x W^T: shared-stationary f32r matmuls + DVE 32x32 block
     transpose between the two contractions. Kinds (q,k,v) on partition
     strips 0-31/32-63/64-95.
  B. logm via inverse scaling-squaring (2 scaled Newton-Schulz sqrt stages +
     degree-11 log series), NORMALIZED form: per-round scalars applied on the
     PSUM->SBUF copies so iterates stay O(1). Mixed precision:
       - Q/K chains: fully fp16 (attention path tolerates it; ~5e-4 full-pipe
         error), one refinement alpha dropped per stage.
       - V chain: stage-0 first 4 alphas fp32, rest fp16; series fp16.
     QK and V chains emitted as interleaved generators so V matmuls fill PE
     stalls during QK's DVE/Act round-trips (and vice versa).
  C. attention: fp16 Gram via 32 per-j accumulating matmuls, qq/kk via
     f32 square+reduce off the same fp16 lf values, softmax along free axis.
  D. Frechet mean: fp16 matmuls; expm via scaling-squaring with K=2
     squarings (fp32) and degree-12 Taylor (fp16).
"""
import math
import numpy as np

import concourse.bacc as bacc
import concourse.mybir as mybir
import concourse.tile as tile
from concourse.bass_utils import run_bass_kernel_spmd

F32 = mybir.dt.float32
F32R = mybir.dt.float32r
F16 = mybir.dt.float16
MULT = mybir.AluOpType.mult
ADD = mybir.AluOpType.add

B, P, IN = 32, 128, 64
NCORES = 8
BLOC = B // NCORES
GRP = 16
NGRP = P // GRP

CGLOB = 8.5                      # global SPD normalizer, > lambda_max (~7.3)
ALPHAS0 = [1.7939874036898087, 1.6696029929467766, 1.5753856846965621,
           1.3802459084155867, 1.1355312114962206, 1.0145731825395088,
           1.0001600783454123]
ALPHAS1 = [1.639353436157538, 1.3943732234795634, 1.1476361656772485,
           1.0173994934181363, 1.000228417137108]
# log series coeffs on M in [0.16, 0.97]: log(M) ~ sum_k SER[k] (M-I)^k
SER = [0.00025761896563381015, 1.016394391935819, -0.08934176002367167,
       5.76267183490063, 42.59363464146395, 215.5576662374658,
       713.4419495013208, 1577.6143678674662, 2302.793898554353,
       2133.2456306970385, 1137.177063455271, 266.64841671372346]
SDEG = len(SER) - 1              # 11
POW2S = 4.0
LNC_CONST = math.log(CGLOB) + POW2S * SER[0]
EXP_K = 2
EXP_D = 12
EXP_C = [1.0 / math.factorial(k) for k in range(EXP_D + 1)]


def _ns_sched(alphas_pair, split32):
    """Per-stage steps: (gamma, y_scale, z_scale, mm_is_f32).

    Normalized scaled-NS: C = gamma*M + I, psYZ = C @ [Y|Z], copies scaled by
    1.5*a (step 0 additionally folds 1/CGLOB into gamma and the Y copy).
    """
    stages = []
    for st, alphas in enumerate(alphas_pair):
        inv = (1.0 / CGLOB) if st == 0 else 1.0
        steps = []
        for k, a in enumerate(alphas):
            kinv = inv if k == 0 else 1.0
            steps.append((-(a * a / 3.0) * kinv, 1.5 * a * kinv, 1.5 * a,
                          st == 0 and k < split32))
        stages.append(steps)
    return stages


SCHED_QK = _ns_sched((ALPHAS0[:-1], ALPHAS1[:-1]), 0)
SCHED_V = _ns_sched((ALPHAS0, ALPHAS1), 4)

PART_QK = dict(nm="q", strips=(0, 32), lo=0, hi=64, sched=SCHED_QK, klo=0)
PART_V = dict(nm="v", strips=(64,), lo=64, hi=96, sched=SCHED_V, klo=2)


def _ns_gen(nc, ps, gp, mats, lf, ibig, ibig16, ibigl, b, g, part):
    """Generator emitting one NS/series round per next() for one part."""
    gsl = slice(GRP * g, GRP * (g + 1))
    nm, strips = part["nm"], part["strips"]
    lo, hi, sched, klo = part["lo"], part["hi"], part["sched"], part["klo"]
    sl = slice(lo, hi)

    def dt(is32):
        return F32 if is32 else F16

    def ident(is32):
        return (ibig if is32 else ibig16)[sl]

    def stt(out, in0, scl, in1):
        nc.vector.scalar_tensor_tensor(out, in0, float(scl), in1,
                                       op0=MULT, op1=ADD)

    def pmm(out_fn, lhs_fn, rhs_fn):
        for bp in strips:
            for s in range(GRP):
                nc.tensor.matmul(out_fn(bp, s), lhs_fn(bp, s), rhs_fn(bp, s),
                                 start=True, stop=True,
                                 tile_position=(bp, bp))

    first32 = sched[0][0][3]
    mc = gp.tile([96, GRP, 32], dt(first32), tag=f"mc{int(first32)}{nm}")
    nc.vector.tensor_copy(mc[sl], mats[sl, gsl, :])
    yield
    for st, steps in enumerate(sched):
        n = len(steps)
        for k, (gam, ysc, _zsc, is32) in enumerate(steps):
            nxt32 = steps[k + 1][3] if k + 1 < n else (
                sched[st + 1][0][3] if st + 1 < len(sched) else False)
            ct = gp.tile([96, GRP, 32], dt(is32), tag=f"ct{int(is32)}{nm}")
            if k == 0:
                stt(ct[sl], mc[sl], gam, ident(is32))
                psY = ps.tile([96, GRP, 32], F32, tag=f"pT{nm}",
                              name=f"psY{nm}{b}_{g}_{st}")
                pmm(lambda bp, s: psY[bp:bp + 32, s, :],
                    lambda bp, s: ct[bp:bp + 32, s, :],
                    lambda bp, s: mc[bp:bp + 32, s, :])
                yz = gp.tile([96, GRP, 64], dt(nxt32),
                             tag=f"yz{int(nxt32)}{nm}")
                nc.scalar.mul(yz[sl, :, 0:32], psY[sl], float(ysc))
                nc.vector.tensor_scalar_mul(yz[sl, :, 32:64], ct[sl],
                                            float(_zsc))
                yield
                continue
            psT = ps.tile([96, GRP, 32], F32, tag=f"pT{nm}",
                          name=f"psT{nm}{b}_{g}_{st}_{k}")
            pmm(lambda bp, s: psT[bp:bp + 32, s, :],
                lambda bp, s: yz[bp:bp + 32, s, 32:64],
                lambda bp, s: yz[bp:bp + 32, s, 0:32])
            stt(ct[sl], psT[sl], gam, ident(is32))
            psYZ = ps.tile([96, GRP, 64], F32, tag=f"pYZ{nm}",
                           name=f"psYZ{nm}{b}_{g}_{st}_{k}")
            pmm(lambda bp, s: psYZ[bp:bp + 32, s, :],
                lambda bp, s: ct[bp:bp + 32, s, :],
                lambda bp, s: yz[bp:bp + 32, s, :])
            if k == n - 1:
                mc = gp.tile([96, GRP, 32], dt(nxt32),
                             tag=f"mc{int(nxt32)}{nm}")
                nc.scalar.mul(mc[sl], psYZ[sl, :, 0:32], float(ysc))
            else:
                yz2 = gp.tile([96, GRP, 64], dt(nxt32),
                              tag=f"yz{int(nxt32)}{nm}")
                nc.scalar.mul(yz2[sl], psYZ[sl], float(ysc))
                yz = yz2
            yield

    # ---- series (fp16) ----
    et = gp.tile([96, GRP, 32], F16, tag=f"et{nm}")
    stt(et[sl], ibig16[sl], -1.0, mc[sl])
    ac = gp.tile([96, GRP, 32], F16, tag=f"ac{nm}")
    nc.vector.tensor_scalar_mul(ac[sl], ibig16[sl], float(POW2S * SER[SDEG]))
    yield
    for k in range(SDEG - 1, 0, -1):
        psH = ps.tile([96, GRP, 32], F32, tag=f"pT{nm}",
                      name=f"psH{nm}{b}_{g}_{k}")
        pmm(lambda bp, s: psH[bp:bp + 32, s, :],
            lambda bp, s: et[bp:bp + 32, s, :],
            lambda bp, s: ac[bp:bp + 32, s, :])
        stt(ac[sl], ibig16[sl], float(POW2S * SER[k]), psH[sl])
        yield
    nk = len(strips)
    psL = ps.tile([32, nk, GRP, 32], F32,
                  tag=(f"pYZ{nm}" if nk == 2 else f"pT{nm}"),
                  name=f"psL{nm}{b}_{g}")
    for i, bp in enumerate(strips):
        for s in range(GRP):
            nc.tensor.matmul(psL[:, i, s, :], et[bp:bp + 32, s, :],
                             ac[bp:bp + 32, s, :], start=True, stop=True,
                             tile_position=(bp, 0))
    nc.vector.tensor_tensor(lf[:, klo:klo + nk, gsl, :], psL[:],
                            ibigl[:, klo:klo + nk], op=ADD)
    yield


def build_nc(debug_stage=99):
    nc = bacc.Bacc("TRN2", target_bir_lowering=False, debug=False,
                   num_devices=NCORES)
    x_in = nc.dram_tensor("x", [BLOC, P, IN, IN], F32, kind="ExternalInput").ap()
    wallT_in = nc.dram_tensor("wallT", [IN, 96], F32, kind="ExternalInput").ap()
    wall2_in = nc.dram_tensor("wall2", [96, 64], F32, kind="ExternalInput").ap()
    ibig_in = nc.dram_tensor("ibig", [128, GRP, 32], F32, kind="ExternalInput").ap()
    ibigl_in = nc.dram_tensor("ibigl", [32, 3 * GRP * 32], F32, kind="ExternalInput").ap()
    ibgx_in = nc.dram_tensor("ibgx", [128, 32, 32], F32, kind="ExternalInput").ap()
    id128_in = nc.dram_tensor("id128", [128, 128], F32, kind="ExternalInput").ap()
    ones_in = nc.dram_tensor("onesc", [96, 128], F32, kind="ExternalInput").ap()
    out_d = nc.dram_tensor("out", [BLOC, P, 32, 32], F32, kind="ExternalOutput").ap()

    with tile.TileContext(nc) as tc:
        with (
            tc.tile_pool(name="const", bufs=1) as cpool,
            tc.tile_pool(name="perb", bufs=1) as bpool,
            tc.tile_pool(name="grp", bufs=2) as gpool,
            tc.tile_pool(name="ps", bufs=1, space="PSUM") as ps,
            tc.tile_pool(name="dscr", bufs=1, space="DRAM") as dpool,
        ):
            scrV_t = dpool.tile([32, P, 32], F16, name="scrV")
            scrM_t = dpool.tile([P, 1024], F16, name="scrM")
            scrV = scrV_t[:]
            scrM = scrM_t[:]
            wallT = cpool.tile([IN, 96], F32)
            nc.sync.dma_start(wallT[:], wallT_in[:])
            wallTr = cpool.tile([IN, 96], F32R)
            nc.vector.tensor_copy(wallTr[:], wallT[:])
            wall2 = cpool.tile([96, 2, 32], F32)
            nc.sync.dma_start(wall2[:], wall2_in.rearrange("p (h j) -> p h j", h=2))
            ibig = cpool.tile([128, GRP, 32], F32)
            nc.sync.dma_start(ibig[:], ibig_in[:])
            ibig16 = cpool.tile([128, GRP, 32], F16)
            nc.vector.tensor_copy(ibig16[:], ibig[:])
            ibigl = cpool.tile([32, 3, GRP, 32], F32)
            nc.sync.dma_start(ibigl[:], ibigl_in.rearrange(
                "p (w s j) -> p w s j", w=3, s=GRP))
            ibgx = cpool.tile([128, 32, 32], F32)
            nc.sync.dma_start(ibgx[:], ibgx_in[:])
            ibgx16 = cpool.tile([128, 32, 32], F16)
            nc.vector.tensor_copy(ibgx16[:], ibgx[:])
            id128 = cpool.tile([128, 128], F32)
            nc.sync.dma_start(id128[:], id128_in[:])
            onesc = cpool.tile([96, 128], F32)
            nc.sync.dma_start(onesc[:], ones_in[:])
            onesc16 = cpool.tile([96, 128], F16)
            nc.vector.tensor_copy(onesc16[:], onesc[:])

            for b in range(BLOC):
                # ================= stage A =================
                xt = bpool.tile([IN, P, IN], F32, tag="xt", bufs=1)
                nc.sync.dma_start(xt[:], x_in[b].rearrange("p i j -> i p j"))
                ytT = bpool.tile([96, P, 2, 32], F32, tag="ytT", bufs=1)
                for t in range(16):
                    xs = xt[:].rearrange("i p j -> i (p j)")[:, 512 * t:512 * (t + 1)]
                    xr = gpool.tile([IN, 512], F32R, tag="xr")
                    nc.vector.tensor_copy(xr[:], xs)
                    psY = ps.tile([96, 512], F32, tag="pTq", name=f"psYa{b}_{t}")
                    nc.tensor.matmul(psY[:], wallTr[:], xr[:], start=True, stop=True)
                    nc.vector.transpose(
                        ytT[:].rearrange("p m h j -> p (m h j)")[:, 512 * t:512 * (t + 1)],
                        psY[:])
                mats = bpool.tile([96, P, 32], F32, tag="mats", bufs=1)
                for g in range(NGRP):
                    gsl = slice(GRP * g, GRP * (g + 1))
                    psQ = ps.tile([96, GRP, 32], F32, tag="pTq", name=f"psQ{b}_{g}")
                    for h in range(2):
                        for w in range(3):
                            sl = slice(32 * w, 32 * w + 32)
                            nc.tensor.matmul(
                                psQ[sl, :, :], wall2[sl, h, :],
                                ytT[sl, gsl, h, :],
                                start=(h == 0), stop=(h == 1),
                                tile_position=(32 * w, 32 * w))
                    nc.scalar.copy(mats[:, gsl, :], psQ[:])

                # ================= stage B: logm =================
                lf = bpool.tile([32, 3, P, 32], F16, tag="lf", bufs=1, name="lf")
                for g in range(NGRP):
                    gq = _ns_gen(nc, ps, gpool, mats, lf, ibig, ibig16, ibigl,
                                 b, g, PART_QK)
                    gv = _ns_gen(nc, ps, gpool, mats, lf, ibig, ibig16, ibigl,
                                 b, g, PART_V)
                    alive = [gq, gv]
                    while alive:
                        for gen in list(alive):
                            try:
                                next(gen)
                            except StopIteration:
                                alive.remove(gen)

                # ================= stage C: attention =================
                qrow = bpool.tile([1, 128], F16, tag="qrow", bufs=1)
                krow = bpool.tile([1, 128], F16, tag="krow", bufs=1)
                for kind, row in ((0, qrow), (1, krow)):
                    sqf = bpool.tile([32, P, 32], F32, tag="sqf", bufs=1)
                    nc.vector.tensor_tensor(sqf[:], lf[:, kind], lf[:, kind],
                                            op=MULT)
                    rsf = bpool.tile([32, P], F32, tag="rsf", bufs=1)
                    nc.vector.tensor_reduce(rsf[:], sqf[:],
                                            axis=mybir.AxisListType.X, op=ADD)
                    psq = ps.tile([1, 128], F32, tag="pC",
                                  name=f"psq{b}_{kind}")
                    nc.tensor.matmul(psq[:], onesc[0:32, 0:1], rsf[:],
                                     start=True, stop=True)
                    nc.scalar.mul(row[:], psq[:], -0.5)
                psE = ps.tile([128, 128], F32, tag="pC", name=f"psE{b}")
                for j in range(32):
                    nc.tensor.matmul(psE[:], lf[:, 1, :, j], lf[:, 0, :, j],
                                     start=(j == 0), stop=False)
                nc.tensor.matmul(psE[:], onesc16[0:1, :], qrow[:],
                                 start=False, stop=False)
                nc.tensor.matmul(psE[:], krow[:], onesc16[0:1, :],
                                 start=False, stop=True)
                w1 = bpool.tile([128, 128], F32, tag="w1", bufs=1)
                nc.scalar.activation(w1[:], psE[:],
                                     mybir.ActivationFunctionType.Relu,
                                     scale=-2.0)
                w2 = bpool.tile([128, 128], F32, tag="w2", bufs=1)
                nc.scalar.activation(w2[:], w1[:],
                                     mybir.ActivationFunctionType.Ln, bias=1.0)
                nc.vector.tensor_scalar_add(w2[:], w2[:], 1.0)
                wr = bpool.tile([128, 128], F32, tag="wr", bufs=1)
                nc.vector.reciprocal(wr[:], w2[:])
                srow = bpool.tile([128, 1], F32, tag="srow", bufs=1)
                ew = bpool.tile([128, 128], F32, tag="ew", bufs=1)
                nc.scalar.activation(ew[:], wr[:],
                                     mybir.ActivationFunctionType.Exp,
                                     accum_out=srow[:])
                rsrow = bpool.tile([128, 1], F32, tag="rsrow", bufs=1)
                nc.vector.reciprocal(rsrow[:], srow[:])
                stile = bpool.tile([128, 128], F32, tag="stile", bufs=1)
                nc.scalar.mul(stile[:], ew[:], rsrow[:])
                psST = ps.tile([128, 128], F32, tag="pC", name=f"psST{b}")
                nc.tensor.transpose(psST[:], stile[:], id128[:])
                st_t = bpool.tile([128, 128], F16, tag="st_t", bufs=1)
                nc.scalar.copy(st_t[:], psST[:])
                lvfs = bpool.tile([128, 1024], F16, tag="lvfs", bufs=1)
                nc.sync.dma_start(scrV[:], lf[:, 2])
                nc.sync.dma_start(
                    lvfs[:].rearrange("m (i j) -> m i j", i=32),
                    scrV.rearrange("i m j -> m i j"))
                psML = ps.tile([128, 8, 128], F32, tag="pYZq", name=f"psML{b}")
                for c in range(8):
                    nc.tensor.matmul(psML[:, c, :], lvfs[:, 128 * c:128 * (c + 1)],
                                     st_t[:], start=True, stop=True)
                mlT = bpool.tile([128, 8, 128], F32, tag="mlT", bufs=1)
                nc.scalar.copy(mlT[:], psML[:])
                psMT = ps.tile([128, 8, 128], F32, tag="pYZv", name=f"psMT{b}")
                for c in range(8):
                    nc.tensor.transpose(psMT[:, c, :], mlT[:, c, :], id128[:])
                mlfs = bpool.tile([128, 1024], F16, tag="mlfs", bufs=1)
                nc.scalar.mul(mlfs[:], psMT[:].rearrange("m c e -> m (c e)"),
                              1.0 / (2.0 ** EXP_K))

                # ================= stage D: expm =================
                gt = bpool.tile([128, 32, 32], F16, tag="gt", bufs=1)
                nc.sync.dma_start(scrM[:], mlfs[:])
                for rr in range(4):
                    nc.sync.dma_start(
                        gt[32 * rr:32 * rr + 32, :, :],
                        scrM[rr::4, :].rearrange("g (i j) -> i g j", i=32))
                acx = bpool.tile([128, 32, 32], F16, tag="acx", bufs=1)
                acx32 = bpool.tile([128, 32, 32], F32, tag="acx32", bufs=1)
                nc.vector.tensor_scalar_mul(acx[:], ibgx16[:], float(EXP_C[EXP_D]))
                for k in range(EXP_D - 1, -1, -1):
                    psX = ps.tile([128, 32, 32], F32, tag="pYZq",
                                  name=f"psXh{b}_{k}")
                    for r in range(4):
                        sl = slice(32 * r, 32 * r + 32)
                        for s in range(32):
                            nc.tensor.matmul(psX[sl, s, :], gt[sl, s, :],
                                             acx[sl, s, :], start=True, stop=True,
                                             tile_position=(32 * r, 32 * r))
                    if k == 0:
                        nc.vector.scalar_tensor_tensor(
                            acx32[:], ibgx[:], float(EXP_C[k]), psX[:],
                            op0=MULT, op1=ADD)
                    else:
                        nc.vector.scalar_tensor_tensor(
                            acx[:], ibgx16[:], float(EXP_C[k]), psX[:],
                            op0=MULT, op1=ADD)
                for sq_i in range(EXP_K):
                    psX = ps.tile([128, 32, 32], F32, tag="pYZq",
                                  name=f"psXs{b}_{sq_i}")
                    for r in range(4):
                        sl = slice(32 * r, 32 * r + 32)
                        for s in range(32):
                            nc.tensor.matmul(psX[sl, s, :], acx32[sl, s, :],
                                             acx32[sl, s, :], start=True, stop=True,
                                             tile_position=(32 * r, 32 * r))
                    nc.scalar.copy(acx32[:], psX[:])
                for rr in range(4):
                    nc.sync.dma_start(
                        out_d[b][rr::4].rearrange("g i j -> i g j"),
                        acx32[32 * rr:32 * rr + 32, :, :])
    nc.compile()
    return nc, {}


def host_constants(Wq, Wk, Wv):
    wallT = np.concatenate([Wq.T, Wk.T, Wv.T], axis=1).astype(np.float32)
    def w2(W):
        WT = np.ascontiguousarray(W.T.astype(np.float32))
        return np.concatenate([WT[0:32], WT[32:64]], axis=1)
    wall2 = np.concatenate([w2(Wq), w2(Wk), w2(Wv)], axis=0)
    eye = np.eye(32, dtype=np.float32)
    ibig = np.broadcast_to(eye[None, :, None, :],
                           (4, 32, GRP, 32)).reshape(128, GRP, 32).copy()
    ibgx = np.broadcast_to(eye[None, :, None, :],
                           (4, 32, 32, 32)).reshape(128, 32, 32).copy()
    ibigl = (LNC_CONST * np.broadcast_to(
        eye[:, None, None, :], (32, 3, GRP, 32))).reshape(32, 3 * GRP * 32)
    ibigl = np.ascontiguousarray(ibigl, dtype=np.float32)
    id128 = np.eye(128, dtype=np.float32)
    onesc = np.ones((96, 128), dtype=np.float32)
    return {"wallT": wallT, "wall2": wall2, "ibig": ibig, "ibgx": ibgx,
            "ibigl": ibigl, "id128": id128, "onesc": onesc}


_NC_CACHE = {}

def kernel(x, Wq, Wk, Wv):
    if "full" not in _NC_CACHE:
        _NC_CACHE["full"] = build_nc(99)
    nc, _ = _NC_CACHE["full"]
    consts = host_constants(np.asarray(Wq), np.asarray(Wk), np.asarray(Wv))
    x = np.asarray(x, dtype=np.float32)
    in_maps = []
    for c in range(NCORES):
        m = {"x": np.ascontiguousarray(x[BLOC * c:BLOC * (c + 1)])}
        m.update(consts)
        in_maps.append(m)
    res = run_bass_kernel_spmd(nc, in_maps, list(range(NCORES)))
    out = np.concatenate([res.results[c]["out"] for c in range(NCORES)], axis=0)
    return out.astype(np.float32)


# revision 12
# speedup vs baseline: 92.9099x; 90.0186x over previous
"""Trainium2 Bass kernel for nn_AttentionManifold (B=32, P=128, IN=64, OUT=32).

Data-parallel over batch: each of 8 NeuronCores handles 4 batches.
Per core:
  A. Q/K/V = W x W^T: shared-stationary f32r matmuls + DVE 32x32 block
     transpose between the two contractions. Kinds (q,k,v) on partition
     strips 0-31/32-63/64-95.
  B. logm via inverse scaling-squaring (2 scaled Newton-Schulz sqrt stages +
     degree-11 log series), NORMALIZED form: per-round scalars applied on the
     PSUM->SBUF copies so iterates stay O(1). Mixed precision:
       - Q/K chains: fully fp16 (attention path tolerates it; ~5e-4 full-pipe
         error), one refinement alpha dropped per stage.
       - V chain: stage-0 first 4 alphas fp32, rest fp16; series fp16.
     QK and V chains emitted as interleaved generators so V matmuls fill PE
     stalls during QK's DVE/Act round-trips (and vice versa).
  C. attention: fp16 Gram via 32 per-j accumulating matmuls, qq/kk via
     f32 square+reduce off the same fp16 lf values, softmax along free axis.
  D. Frechet mean: fp16 matmuls; expm via scaling-squaring with K=2
     squarings (fp32) and degree-12 Taylor (fp16).
"""
import math
import numpy as np

import concourse.bacc as bacc
import concourse.mybir as mybir
import concourse.tile as tile
from concourse.bass_utils import run_bass_kernel_spmd

F32 = mybir.dt.float32
F32R = mybir.dt.float32r
F16 = mybir.dt.float16
MULT = mybir.AluOpType.mult
ADD = mybir.AluOpType.add

B, P, IN = 32, 128, 64
NCORES = 8
BLOC = B // NCORES
GRP = 16
NGRP = P // GRP

CGLOB = 8.5                      # global SPD normalizer, > lambda_max (~7.3)
ALPHAS0 = [1.7939874036898087, 1.6696029929467766, 1.5753856846965621,
           1.3802459084155867, 1.1355312114962206, 1.0145731825395088,
           1.0001600783454123]
ALPHAS1 = [1.639353436157538, 1.3943732234795634, 1.1476361656772485,
           1.0173994934181363, 1.000228417137108]
# log series coeffs on M in [0.16, 0.97]: log(M) ~ sum_k SER[k] (M-I)^k
SER = [0.00025761896563381015, 1.016394391935819, -0.08934176002367167,
       5.76267183490063, 42.59363464146395, 215.5576662374658,
       713.4419495013208, 1577.6143678674662, 2302.793898554353,
       2133.2456306970385, 1137.177063455271, 266.64841671372346]
SDEG = len(SER) - 1              # 11
POW2S = 4.0
LNC_CONST = math.log(CGLOB) + POW2S * SER[0]
EXP_K = 2
EXP_D = 12
EXP_C = [1.0 / math.factorial(k) for k in range(EXP_D + 1)]


def _ns_sched(alphas_pair, split32):
    """Per-stage steps: (gamma, y_scale, z_scale, mm_is_f32).

    Normalized scaled-NS: C = gamma*M + I, psYZ = C @ [Y|Z], copies scaled by
    1.5*a (step 0 additionally folds 1/CGLOB into gamma and the Y copy).
    """
    stages = []
    for st, alphas in enumerate(alphas_pair):
        inv = (1.0 / CGLOB) if st == 0 else 1.0
        steps = []
        for k, a in enumerate(alphas):
            kinv = inv if k == 0 else 1.0
            steps.append((-(a * a / 3.0) * kinv, 1.5 * a * kinv, 1.5 * a,
                          st == 0 and k < split32))
        stages.append(steps)
    return stages


SCHED_QK = _ns_sched((ALPHAS0[:-1], ALPHAS1[:-1]), 0)
SCHED_V = _ns_sched((ALPHAS0, ALPHAS1), 4)

PART_QK = dict(nm="q", strips=(0, 32), lo=0, hi=64, sched=SCHED_QK, klo=0)
PART_V = dict(nm="v", strips=(64,), lo=64, hi=96, sched=SCHED_V, klo=2)


def _ns_gen(nc, ps, gp, mats, lf, ibig, ibig16, ibigl, b, g, part):
    """Generator emitting one NS/series round per next() for one part."""
    gsl = slice(GRP * g, GRP * (g + 1))
    nm, strips = part["nm"], part["strips"]
    lo, hi, sched, klo = part["lo"], part["hi"], part["sched"], part["klo"]
    sl = slice(lo, hi)

    def dt(is32):
        return F32 if is32 else F16

    def ident(is32):
        return (ibig if is32 else ibig16)[sl]

    def stt(out, in0, scl, in1):
        nc.vector.scalar_tensor_tensor(out, in0, float(scl), in1,
                                       op0=MULT, op1=ADD)

    def pmm(out_fn, lhs_fn, rhs_fn):
        for bp in strips:
            for s in range(GRP):
                nc.tensor.matmul(out_fn(bp, s), lhs_fn(bp, s), rhs_fn(bp, s),
                                 start=True, stop=True,
                                 tile_position=(bp, bp))

    first32 = sched[0][0][3]
    mc = gp.tile([96, GRP, 32], dt(first32), tag=f"mc{int(first32)}{nm}")
    nc.vector.tensor_copy(mc[sl], mats[sl, gsl, :])
    yield
    for st, steps in enumerate(sched):
        n = len(steps)
        for k, (gam, ysc, _zsc, is32) in enumerate(steps):
            nxt32 = steps[k + 1][3] if k + 1 < n else (
                sched[st + 1][0][3] if st + 1 < len(sched) else False)
            ct = gp.tile([96, GRP, 32], dt(is32), tag=f"ct{int(is32)}{nm}")
            if k == 0:
                stt(ct[sl], mc[sl], gam, ident(is32))
                psY = ps.tile([96, GRP, 32], F32, tag=f"pT{nm}",
                              name=f"psY{nm}{b}_{g}_{st}")
                pmm(lambda bp, s: psY[bp:bp + 32, s, :],
                    lambda bp, s: ct[bp:bp + 32, s, :],
                    lambda bp, s: mc[bp:bp + 32, s, :])
                yz = gp.tile([96, GRP, 64], dt(nxt32),
                             tag=f"yz{int(nxt32)}{nm}")
                nc.scalar.mul(yz[sl, :, 0:32], psY[sl], float(ysc))
                nc.vector.tensor_scalar_mul(yz[sl, :, 32:64], ct[sl],
                                            float(_zsc))
                yield
                continue
            psT = ps.tile([96, GRP, 32], F32, tag=f"pT{nm}",
                          name=f"psT{nm}{b}_{g}_{st}_{k}")
            pmm(lambda bp, s: psT[bp:bp + 32, s, :],
                lambda bp, s: yz[bp:bp + 32, s, 32:64],
                lambda bp, s: yz[bp:bp + 32, s, 0:32])
            stt(ct[sl], psT[sl], gam, ident(is32))
            psYZ = ps.tile([96, GRP, 64], F32, tag=f"pYZ{nm}",
                           name=f"psYZ{nm}{b}_{g}_{st}_{k}")
            pmm(lambda bp, s: psYZ[bp:bp + 32, s, :],
                lambda bp, s: ct[bp:bp + 32, s, :],
                lambda bp, s: yz[bp:bp + 32, s, :])
            if k == n - 1:
                mc = gp.tile([96, GRP, 32], dt(nxt32),
                             tag=f"mc{int(nxt32)}{nm}")
                nc.scalar.mul(mc[sl], psYZ[sl, :, 0:32], float(ysc))
            else:
                yz2 = gp.tile([96, GRP, 64], dt(nxt32),
                              tag=f"yz{int(nxt32)}{nm}")
                nc.scalar.mul(yz2[sl], psYZ[sl], float(ysc))
                yz = yz2
            yield

    # ---- series (fp16, Horner) ----
    et = gp.tile([96, GRP, 32], F16, tag=f"et{nm}")
    stt(et[sl], ibig16[sl], -1.0, mc[sl])
    ac = gp.tile([96, GRP, 32], F16, tag=f"ac{nm}")
    nc.vector.tensor_scalar_mul(ac[sl], ibig16[sl], float(POW2S * SER[SDEG]))
    yield
    for k in range(SDEG - 1, 0, -1):
        psH = ps.tile([96, GRP, 32], F32, tag=f"pT{nm}",
                      name=f"psH{nm}{b}_{g}_{k}")
        pmm(lambda bp, s: psH[bp:bp + 32, s, :],
            lambda bp, s: et[bp:bp + 32, s, :],
            lambda bp, s: ac[bp:bp + 32, s, :])
        stt(ac[sl], ibig16[sl], float(POW2S * SER[k]), psH[sl])
        yield
    nk = len(strips)
    psL = ps.tile([32, nk, GRP, 32], F32,
                  tag=(f"pYZ{nm}" if nk == 2 else f"pT{nm}"),
                  name=f"psL{nm}{b}_{g}")
    for i, bp in enumerate(strips):
        for s in range(GRP):
            nc.tensor.matmul(psL[:, i, s, :], et[bp:bp + 32, s, :],
                             ac[bp:bp + 32, s, :], start=True, stop=True,
                             tile_position=(bp, 0))
    nc.vector.tensor_tensor(lf[:, klo:klo + nk, gsl, :], psL[:],
                            ibigl[:, klo:klo + nk], op=ADD)
    yield


def build_nc(debug_stage=99):
    nc = bacc.Bacc("TRN2", target_bir_lowering=False, debug=False,
                   num_devices=NCORES)
    x_in = nc.dram_tensor("x", [BLOC, P, IN, IN], F32R, kind="ExternalInput").ap()
    wallT_in = nc.dram_tensor("wallT", [IN, 96], F32, kind="ExternalInput").ap()
    wall2_in = nc.dram_tensor("wall2", [96, 64], F32, kind="ExternalInput").ap()
    ibig_in = nc.dram_tensor("ibig", [128, GRP, 32], F32, kind="ExternalInput").ap()
    ibigl_in = nc.dram_tensor("ibigl", [32, 3 * GRP * 32], F32, kind="ExternalInput").ap()
    ibgx_in = nc.dram_tensor("ibgx", [128, 32, 32], F32, kind="ExternalInput").ap()
    id128_in = nc.dram_tensor("id128", [128, 128], F32, kind="ExternalInput").ap()
    ones_in = nc.dram_tensor("onesc", [96, 128], F32, kind="ExternalInput").ap()
    out_d = nc.dram_tensor("out", [BLOC, P, 32, 32], F32, kind="ExternalOutput").ap()

    with tile.TileContext(nc) as tc:
        with (
            tc.tile_pool(name="const", bufs=1) as cpool,
            tc.tile_pool(name="perb", bufs=1) as bpool,
            tc.tile_pool(name="grp", bufs=2) as gpool,
            tc.tile_pool(name="ps", bufs=1, space="PSUM") as ps,
            tc.tile_pool(name="dscr", bufs=1, space="DRAM") as dpool,
        ):
            scrV_t = dpool.tile([32, P, 32], F16, name="scrV")
            scrM_t = dpool.tile([P, 1024], F16, name="scrM")
            scrV = scrV_t[:]
            scrM = scrM_t[:]
            wallT = cpool.tile([IN, 96], F32)
            nc.sync.dma_start(wallT[:], wallT_in[:])
            wallTr = cpool.tile([IN, 96], F32R)
            nc.vector.tensor_copy(wallTr[:], wallT[:])
            wall2 = cpool.tile([96, 2, 32], F32)
            nc.sync.dma_start(wall2[:], wall2_in.rearrange("p (h j) -> p h j", h=2))
            ibig = cpool.tile([128, GRP, 32], F32)
            nc.sync.dma_start(ibig[:], ibig_in[:])
            ibig16 = cpool.tile([128, GRP, 32], F16)
            nc.vector.tensor_copy(ibig16[:], ibig[:])
            ibigl = cpool.tile([32, 3, GRP, 32], F32)
            nc.sync.dma_start(ibigl[:], ibigl_in.rearrange(
                "p (w s j) -> p w s j", w=3, s=GRP))
            ibgx = cpool.tile([128, 32, 32], F32)
            nc.sync.dma_start(ibgx[:], ibgx_in[:])
            ibgx16 = cpool.tile([128, 32, 32], F16)
            nc.vector.tensor_copy(ibgx16[:], ibgx[:])
            id128 = cpool.tile([128, 128], F32)
            nc.sync.dma_start(id128[:], id128_in[:])
            onesc = cpool.tile([96, 128], F32)
            nc.sync.dma_start(onesc[:], ones_in[:])
            onesc16 = cpool.tile([96, 128], F16)
            nc.vector.tensor_copy(onesc16[:], onesc[:])

            for b in range(BLOC):
                # ================= stage A =================
                xt = bpool.tile([IN, P, IN], F32R, tag="xt", bufs=1)
                nc.sync.dma_start(xt[:], x_in[b].rearrange("p i j -> i p j"))
                ytTr = bpool.tile([96, P, 2, 32], F32, tag="ytTr", bufs=1)
                for t in range(16):
                    xs = xt[:].rearrange("i p j -> i (p j)")[:, 512 * t:512 * (t + 1)]
                    psY = ps.tile([96, 512], F32, tag="pTq", name=f"psYa{b}_{t}")
                    nc.tensor.matmul(psY[:], wallTr[:], xs, start=True, stop=True)
                    yts = gpool.tile([96, 512], F32, tag="ytstg")
                    nc.vector.transpose(yts[:], psY[:])
                    nc.vector.tensor_copy(
                        ytTr[:].rearrange("p m h j -> p (m h j)")[:, 512 * t:512 * (t + 1)],
                        yts[:])
                mats = bpool.tile([96, P, 32], F32, tag="mats", bufs=1)
                for g in range(NGRP):
                    gsl = slice(GRP * g, GRP * (g + 1))
                    psQ = ps.tile([96, GRP, 32], F32, tag="pTq", name=f"psQ{b}_{g}")
                    for h in range(2):
                        for w in range(3):
                            sl = slice(32 * w, 32 * w + 32)
                            nc.tensor.matmul(
                                psQ[sl, :, :], wall2[sl, h, :],
                                ytTr[sl, gsl, h, :],
                                start=(h == 0), stop=(h == 1),
                                tile_position=(32 * w, 32 * w))
                    nc.scalar.copy(mats[:, gsl, :], psQ[:])

                # ================= stage B: logm =================
                lf = bpool.tile([32, 3, P, 32], F16, tag="lf", bufs=1, name="lf")
                for g in range(NGRP):
                    gq = _ns_gen(nc, ps, gpool, mats, lf, ibig, ibig16, ibigl,
                                 b, g, PART_QK)
                    gv = _ns_gen(nc, ps, gpool, mats, lf, ibig, ibig16, ibigl,
                                 b, g, PART_V)
                    alive = [gq, gv]
                    while alive:
                        for gen in list(alive):
                            try:
                                next(gen)
                            except StopIteration:
                                alive.remove(gen)

                # ================= stage C: attention =================
                qrow = bpool.tile([1, 128], F16, tag="qrow", bufs=1)
                krow = bpool.tile([1, 128], F16, tag="krow", bufs=1)
                for kind, row in ((0, qrow), (1, krow)):
                    sqf = bpool.tile([32, P, 32], F32, tag="ytTr", bufs=1)
                    nc.vector.tensor_tensor(sqf[:], lf[:, kind], lf[:, kind],
                                            op=MULT)
                    rsf = bpool.tile([32, P], F32, tag="rsf", bufs=1)
                    nc.vector.tensor_reduce(rsf[:], sqf[:],
                                            axis=mybir.AxisListType.X, op=ADD)
                    psq = ps.tile([1, 128], F32, tag="pC",
                                  name=f"psq{b}_{kind}")
                    nc.tensor.matmul(psq[:], onesc[0:32, 0:1], rsf[:],
                                     start=True, stop=True)
                    nc.scalar.mul(row[:], psq[:], -0.5)
                psE = ps.tile([128, 128], F32, tag="pC", name=f"psE{b}")
                for j in range(32):
                    nc.tensor.matmul(psE[:], lf[:, 1, :, j], lf[:, 0, :, j],
                                     start=(j == 0), stop=False)
                nc.tensor.matmul(psE[:], onesc16[0:1, :], qrow[:],
                                 start=False, stop=False)
                nc.tensor.matmul(psE[:], krow[:], onesc16[0:1, :],
                                 start=False, stop=True)
                w1 = bpool.tile([128, 128], F32, tag="w1", bufs=1)
                nc.scalar.activation(w1[:], psE[:],
                                     mybir.ActivationFunctionType.Relu,
                                     scale=-2.0)
                w2 = bpool.tile([128, 128], F32, tag="w2", bufs=1)
                nc.scalar.activation(w2[:], w1[:],
                                     mybir.ActivationFunctionType.Ln, bias=1.0)
                nc.vector.tensor_scalar_add(w2[:], w2[:], 1.0)
                wr = bpool.tile([128, 128], F32, tag="wr", bufs=1)
                nc.vector.reciprocal(wr[:], w2[:])
                srow = bpool.tile([128, 1], F32, tag="srow", bufs=1)
                ew = bpool.tile([128, 128], F32, tag="ew", bufs=1)
                nc.scalar.activation(ew[:], wr[:],
                                     mybir.ActivationFunctionType.Exp,
                                     accum_out=srow[:])
                rsrow = bpool.tile([128, 1], F32, tag="rsrow", bufs=1)
                nc.vector.reciprocal(rsrow[:], srow[:])
                stile = bpool.tile([128, 128], F32, tag="stile", bufs=1)
                nc.scalar.mul(stile[:], ew[:], rsrow[:])
                psST = ps.tile([128, 128], F32, tag="pC", name=f"psST{b}")
                nc.tensor.transpose(psST[:], stile[:], id128[:])
                st_t = bpool.tile([128, 128], F16, tag="st_t", bufs=1)
                nc.scalar.copy(st_t[:], psST[:])
                lvfs = bpool.tile([128, 1024], F16, tag="lvfs", bufs=1)
                nc.sync.dma_start(scrV[:], lf[:, 2])
                nc.sync.dma_start(
                    lvfs[:].rearrange("m (i j) -> m i j", i=32),
                    scrV.rearrange("i m j -> m i j"))
                psML = ps.tile([128, 8, 128], F32, tag="pYZq", name=f"psML{b}")
                for c in range(8):
                    nc.tensor.matmul(psML[:, c, :], lvfs[:, 128 * c:128 * (c + 1)],
                                     st_t[:], start=True, stop=True)
                mlT = bpool.tile([128, 8, 128], F32, tag="mlT", bufs=1)
                nc.scalar.copy(mlT[:], psML[:])
                psMT = ps.tile([128, 8, 128], F32, tag="pYZv", name=f"psMT{b}")
                for c in range(8):
                    nc.tensor.transpose(psMT[:, c, :], mlT[:, c, :], id128[:])
                mlfs = bpool.tile([128, 1024], F16, tag="mlfs", bufs=1)
                nc.scalar.mul(mlfs[:], psMT[:].rearrange("m c e -> m (c e)"),
                              1.0 / (2.0 ** EXP_K))

                # ================= stage D: expm =================
                gt = bpool.tile([128, 32, 32], F16, tag="gt", bufs=1)
                nc.sync.dma_start(scrM[:], mlfs[:])
                for rr in range(4):
                    nc.sync.dma_start(
                        gt[32 * rr:32 * rr + 32, :, :],
                        scrM[rr::4, :].rearrange("g (i j) -> i g j", i=32))
                acx = bpool.tile([128, 32, 32], F16, tag="acx", bufs=1)
                acx32 = bpool.tile([128, 32, 32], F32, tag="acx32", bufs=1)
                nc.vector.tensor_scalar_mul(acx[:], ibgx16[:], float(EXP_C[EXP_D]))
                for k in range(EXP_D - 1, -1, -1):
                    psX = ps.tile([128, 32, 32], F32, tag="pYZq",
                                  name=f"psXh{b}_{k}")
                    for r in range(4):
                        sl = slice(32 * r, 32 * r + 32)
                        for s in range(32):
                            nc.tensor.matmul(psX[sl, s, :], gt[sl, s, :],
                                             acx[sl, s, :], start=True, stop=True,
                                             tile_position=(32 * r, 32 * r))
                    if k == 0:
                        nc.vector.scalar_tensor_tensor(
                            acx32[:], ibgx[:], float(EXP_C[k]), psX[:],
                            op0=MULT, op1=ADD)
                    else:
                        nc.vector.scalar_tensor_tensor(
                            acx[:], ibgx16[:], float(EXP_C[k]), psX[:],
                            op0=MULT, op1=ADD)
                for sq_i in range(EXP_K):
                    psX = ps.tile([128, 32, 32], F32, tag="pYZq",
                                  name=f"psXs{b}_{sq_i}")
                    for r in range(4):
                        sl = slice(32 * r, 32 * r + 32)
                        for s in range(32):
                            nc.tensor.matmul(psX[sl, s, :], acx32[sl, s, :],
                                             acx32[sl, s, :], start=True, stop=True,
                                             tile_position=(32 * r, 32 * r))
                    nc.scalar.copy(acx32[:], psX[:])
                for rr in range(4):
                    nc.sync.dma_start(
                        out_d[b][rr::4].rearrange("g i j -> i g j"),
                        acx32[32 * rr:32 * rr + 32, :, :])
    nc.compile()
    return nc, {}


def host_constants(Wq, Wk, Wv):
    wallT = np.concatenate([Wq.T, Wk.T, Wv.T], axis=1).astype(np.float32)
    def w2(W):
        WT = np.ascontiguousarray(W.T.astype(np.float32))
        return np.concatenate([WT[0:32], WT[32:64]], axis=1)
    wall2 = np.concatenate([w2(Wq), w2(Wk), w2(Wv)], axis=0)
    eye = np.eye(32, dtype=np.float32)
    ibig = np.broadcast_to(eye[None, :, None, :],
                           (4, 32, GRP, 32)).reshape(128, GRP, 32).copy()
    ibgx = np.broadcast_to(eye[None, :, None, :],
                           (4, 32, 32, 32)).reshape(128, 32, 32).copy()
    ibigl = (LNC_CONST * np.broadcast_to(
        eye[:, None, None, :], (32, 3, GRP, 32))).reshape(32, 3 * GRP * 32)
    ibigl = np.ascontiguousarray(ibigl, dtype=np.float32)
    id128 = np.eye(128, dtype=np.float32)
    onesc = np.ones((96, 128), dtype=np.float32)
    return {"wallT": wallT, "wall2": wall2, "ibig": ibig, "ibgx": ibgx,
            "ibigl": ibigl, "id128": id128, "onesc": onesc}


_NC_CACHE = {}

def kernel(x, Wq, Wk, Wv):
    if "full" not in _NC_CACHE:
        _NC_CACHE["full"] = build_nc(99)
    nc, _ = _NC_CACHE["full"]
    consts = host_constants(np.asarray(Wq), np.asarray(Wk), np.asarray(Wv))
    x = np.asarray(x, dtype=np.float32)
    in_maps = []
    for c in range(NCORES):
        m = {"x": np.ascontiguousarray(x[BLOC * c:BLOC * (c + 1)])}
        m.update(consts)
        in_maps.append(m)
    res = run_bass_kernel_spmd(nc, in_maps, list(range(NCORES)))
    out = np.concatenate([res.results[c]["out"] for c in range(NCORES)], axis=0)
    return out.astype(np.float32)
